# revision 16
# baseline (speedup 1.0000x reference)
"""Attention-GRU decoder (nn_GRU_51376398794979) on 8 Trainium2 NeuronCores.

Strategy (per core, data-parallel over batch B=128 -> BL=16):
  - Precompute Uk = e_all @ Ua^T + (bu+ba) on PE, kept resident in SBUF (bf16,
    d-on-partitions layout). e_all kept resident s-on-partitions (bf16).
  - 64 sequential decoder steps, fully unrolled:
      qT = Wa h^T               (PE, bf16, d-on-partitions out)
      arg = Uk + qT             (DVE tensor_scalar, per-partition scalar)
      tanh(arg)                 (ACT, batched [128, 8*512] instructions)
      scores = va^T tanh        (PE, M=4 col-tiled blocks at partitions 32j)
      softmax per group         (DVE max / ACT exp+accum / DVE recip+mul)
      wT via PE transpose       (zero-interleaved lhsT for ctx)
      ctx = w^T e_all           (PE, M=16 single accumulation group)
      GRU gates                 (PE matmuls feature-on-partition + DVE/ACT)
  - cross_attn written per step (bf16), outputs gathered on host.

All matmuls run in bf16 with fp32 PSUM accumulation; softmax and the GRU
state stay fp32. Verified vs the fp32 reference: resid_var ~1e-5 per output.
"""
import numpy as np
import ml_dtypes

try:
    import jax
    jax.config.update("jax_compilation_cache_dir", "/root/.cache/jax_bass_cache")
    jax.config.update("jax_persistent_cache_min_compile_time_secs", 0)
except Exception:
    pass

import concourse.bass as bass
import concourse.tile as tile
from concourse import mybir
from concourse.bass_utils import run_bass_kernel_spmd
from concourse.vector_clock import ScopedClock

F32 = mybir.dt.float32
BF16 = mybir.dt.bfloat16
AF = mybir.ActivationFunctionType
BF = ml_dtypes.bfloat16

B, S, D, T, OUT = 128, 512, 512, 64, 3
NC_ = 8
BL = B // NC_          # 16 batch rows per core
DC = D // 128          # 4 d-chunks
SC = S // 128          # 4 s-chunks
GC = (3 * D) // 128    # 12 gate chunks


# ---------------------------------------------------------------------------
# walrus workaround: this compiler build rejects Drain instructions carrying
# multiple ge-imm sem waits; re-emit the tile final-drain waits as standalone
# wait_ge instructions.
def _patch_tile_drain():
    def _split_multi_waits(self):
        # this walrus build accepts at most ONE sem wait per instruction;
        # hoist extras into standalone per-engine wait_ge instructions.
        nc = self.nc
        id2h = {h.num: h for h in self.sems.allocated().values()}
        for f in nc.m.functions:
            for bb in f.blocks:
                original = [i for i in bb.instructions
                            if i.sync_info and i.sync_info.on_wait
                            and len(i.sync_info.on_wait) > 1]
                for inst in original:
                    waits = list(inst.sync_info.on_wait)
                    ge = [w for w in waits if w.wait_mode == "sem-ge-imm"]
                    other = [w for w in waits if w.wait_mode != "sem-ge-imm"]
                    assert len(other) <= 1, [w.wait_mode for w in waits]
                    if other:
                        keep, split = other, ge
                    else:
                        keep, split = ge[-1:], ge[:-1]
                    movers = []
                    for w in split:
                        h = id2h.get(w.id)
                        assert h is not None, f"no sem handle {w.id} {w.ant_name}"
                        wi = nc.engines[inst.engine].wait_ge(h, w.wait_value)
                        mi = wi.ins if hasattr(wi, "ins") else wi
                        nc.cur_bb.bb.instructions.remove(mi)
                        movers.append(mi)
                    idx = bb.instructions.index(inst)
                    for k, mi in enumerate(movers):
                        bb.instructions.insert(idx + k, mi)
                    inst.sync_info.on_wait = keep

    def _patched(self, tick_clock, wait_clock):
        nc = self.nc
        dummy = nc.sync.drain()
        wait_clock.add_sem_waits(
            dummy.ins, ScopedClock({None: tick_clock.global_clock})
        )
        waits = list(dummy.ins.sync_info.on_wait or []) if dummy.ins.sync_info else []
        nc.cur_bb.bb.instructions.remove(dummy.ins)
        id2h = {h.num: h for h in self.sems.allocated().values()}
        for w in waits:
            h = id2h.get(w.id)
            assert h is not None, f"no handle for sem {w.id} ({w.ant_name})"
            assert w.wait_mode == "sem-ge-imm", w.wait_mode
            nc.sync.wait_ge(h, w.wait_value)
        _split_multi_waits(self)
        nc.all_engine_barrier()
        popped = nc._tile_sem_poison_stack.pop()
        assert popped is self._sem_poison
        nc.clear_and_free_semaphores(list(self.sems.allocated().values()))
        nc.all_engine_barrier()

    tile.TileContext._drain_and_barrier = _patched


_patch_tile_drain()


def build_kernel(t_steps=T, stop_after=None):
    import contextlib
    nc = bass.Bass("TRN2", num_devices=1)

    # ---- DRAM I/O (per-core shard) ----
    eT_d = nc.dram_tensor("eT", [D, BL, S], BF16, kind="ExternalInput")
    eS_d = nc.dram_tensor("eS", [S, BL, D], BF16, kind="ExternalInput")
    xT_d = nc.dram_tensor("xT", [4, T * BL], BF16, kind="ExternalInput")
    h0T_d = nc.dram_tensor("h0T", [D, BL], F32, kind="ExternalInput")
    WaT_d = nc.dram_tensor("WaT", [D, D], BF16, kind="ExternalInput")
    UaT_d = nc.dram_tensor("UaT", [D, D], BF16, kind="ExternalInput")
    WihT_d = nc.dram_tensor("WihT", [D, 3 * D], BF16, kind="ExternalInput")
    WihX_d = nc.dram_tensor("WihX", [4, 3 * D], BF16, kind="ExternalInput")
    WhhT_d = nc.dram_tensor("WhhT", [D, 3 * D], BF16, kind="ExternalInput")
    bhh_d = nc.dram_tensor("bhh", [1, 3 * D], BF16, kind="ExternalInput")
    va_d = nc.dram_tensor("vap", [128, DC, 4, 4], BF16, kind="ExternalInput")
    idT4_d = nc.dram_tensor("idT4", [128, 4], BF16, kind="ExternalInput")
    id16_d = nc.dram_tensor("id16", [16, 16], BF16, kind="ExternalInput")
    WoT_d = nc.dram_tensor("WoT", [D, OUT], BF16, kind="ExternalInput")
    bo_d = nc.dram_tensor("bo", [OUT, 1], F32, kind="ExternalInput")
    buk_d = nc.dram_tensor("buk", [128, DC], F32, kind="ExternalInput")

    attn_d = nc.dram_tensor("attn", [T, BL, S], BF16, kind="ExternalOutput")
    outsT_d = nc.dram_tensor("outsT", [OUT, T * BL], F32, kind="ExternalOutput")
    hT_d = nc.dram_tensor("hTf", [128, DC, BL], F32, kind="ExternalOutput")

    with tile.TileContext(nc) as tc:
        with contextlib.ExitStack() as ctx:
            sing = ctx.enter_context(tc.tile_pool(name="sing", bufs=1))
            stage_p = ctx.enter_context(tc.tile_pool(name="stage", bufs=2))
            step_p = ctx.enter_context(tc.tile_pool(name="step", bufs=2))
            ps_sc_p = ctx.enter_context(
                tc.tile_pool(name="ps_sc", bufs=2, space="PSUM"))
            ps_q_p = ctx.enter_context(
                tc.tile_pool(name="ps_q", bufs=1, space="PSUM"))
            ps_ctx_p = ctx.enter_context(
                tc.tile_pool(name="ps_ctx", bufs=2, space="PSUM"))
            ps_g_p = ctx.enter_context(
                tc.tile_pool(name="ps_g", bufs=1, space="PSUM"))
            ps_t_p = ctx.enter_context(
                tc.tile_pool(name="ps_t", bufs=2, space="PSUM"))

            # ---- persistent SBUF ----
            Uk = sing.tile([128, DC, BL, S], BF16)     # 64KB/part
            Ea = sing.tile([128, SC, BL, D], BF16)     # 64KB/part
            WaT = sing.tile([128, DC, D], BF16)
            WihT = sing.tile([128, DC, 3 * D], BF16)
            WihX = sing.tile([4, 3 * D], BF16)
            WhhT = sing.tile([128, DC, 3 * D], BF16)
            bhh = sing.tile([1, 3 * D], BF16)
            va = sing.tile([128, DC, 4, 4], BF16)
            idT4 = sing.tile([128, 4], BF16)
            id16 = sing.tile([16, 16], BF16)
            WoT = sing.tile([128, DC, OUT], BF16)
            bo = sing.tile([OUT, 1], F32)
            buk = sing.tile([128, DC], F32)
            xT = sing.tile([4, T * BL], BF16)
            ones1 = sing.tile([1, BL], BF16)
            wTz = sing.tile([128, SC, BL * BL], BF16)  # zero-interleaved wT
            hT = [sing.tile([128, DC, BL], F32, name=f"hT{i}", tag=f"hT{i}") for i in range(2)]
            hTbf = [sing.tile([128, DC, BL], BF16, name=f"hTbf{i}", tag=f"hTbf{i}") for i in range(2)]
            outs_st = sing.tile([OUT, T * BL], F32)

            # ---- load constants ----
            for t_, d_ in ((WaT, WaT_d), (WihT, WihT_d), (WhhT, WhhT_d)):
                ap = d_.ap().rearrange("(c p) g -> p c g", p=128)
                nc.sync.dma_start(out=t_, in_=ap)
            nc.sync.dma_start(out=WihX, in_=WihX_d.ap())
            nc.sync.dma_start(out=bhh, in_=bhh_d.ap())
            nc.sync.dma_start(out=va, in_=va_d.ap())
            nc.sync.dma_start(out=idT4, in_=idT4_d.ap())
            nc.sync.dma_start(out=id16, in_=id16_d.ap())
            nc.sync.dma_start(out=WoT, in_=WoT_d.ap().rearrange("(c p) o -> p c o", p=128))
            nc.sync.dma_start(out=bo, in_=bo_d.ap())
            nc.sync.dma_start(out=buk, in_=buk_d.ap())
            nc.sync.dma_start(out=xT, in_=xT_d.ap())
            nc.sync.dma_start(out=hT[0], in_=h0T_d.ap().rearrange("(c p) b -> p c b", p=128))
            nc.vector.memset(ones1, 1.0)
            nc.vector.memset(outs_st, 0.0)
            for c in range(SC):
                nc.vector.memset(wTz[:, c, :], 0.0)
            nc.vector.tensor_copy(hTbf[0], hT[0])

            # e_all resident, s-on-partitions: Ea[p, k, b, d] = e[b, 128k+p, d]
            for k in range(SC):
                nc.sync.dma_start(
                    out=Ea[:, k, :, :],
                    in_=eS_d.ap()[128 * k:128 * (k + 1), :, :])

            # ---- precompute Uk = Ua e^T + (bu+ba), d-on-partitions ----
            with contextlib.ExitStack() as pctx:
                pre = pctx.enter_context(tc.tile_pool(name="pre", bufs=1))
                prest = pctx.enter_context(tc.tile_pool(name="prest", bufs=1))

                UaT = pre.tile([128, DC, D], BF16)
                nc.sync.dma_start(
                    out=UaT, in_=UaT_d.ap().rearrange("(c p) g -> p c g", p=128))
                for b in range(BL):
                    et = prest.tile([128, DC, S], BF16)
                    nc.sync.dma_start(
                        out=et, in_=eT_d.ap()[:, b, :].rearrange("(c p) s -> p c s", p=128))
                    for dc in range(DC):
                        ps = ps_sc_p.tile([128, S], F32, name="ps_uk", tag="ps_sc")
                        for ec in range(DC):
                            nc.tensor.matmul(
                                ps, UaT[:, ec, 128 * dc:128 * (dc + 1)],
                                et[:, ec, :],
                                start=(ec == 0), stop=(ec == DC - 1))
                        nc.vector.tensor_scalar_add(
                            Uk[:, dc, b, :], ps, buk[:, dc:dc + 1])

            # ---- decoder steps ----
            for t in range(t_steps):
                cur, nxt = t % 2, (t + 1) % 2
                if stop_after == "nothing":
                    nc.vector.tensor_copy(hT[nxt], hT[cur])
                    nc.vector.tensor_copy(hTbf[nxt], hTbf[cur])
                    continue
                # qT = Wa h^T  -> [128, dc, BL] psum
                ps_q = ps_q_p.tile([128, DC, BL], F32)
                for dc in range(DC):
                    for ec in range(DC):
                        nc.tensor.matmul(
                            ps_q[:, dc, :],
                            WaT[:, ec, 128 * dc:128 * (dc + 1)],
                            hTbf[cur][:, ec, :],
                            start=(ec == 0), stop=(ec == DC - 1))
                qT = step_p.tile([128, DC, BL], F32)
                nc.vector.tensor_copy(qT, ps_q)

                # tanh(Uk + qT) then scores, col-tiled M=4 blocks
                ps_sc = ps_sc_p.tile([128, S], F32)
                for dc in range(DC):
                    for bh in range(2):
                        st = stage_p.tile([128, 8, S], BF16)
                        for i in range(8):
                            b = 8 * bh + i
                            nc.vector.tensor_scalar_add(
                                st[:, i, :], Uk[:, dc, b, :], qT[:, dc, b:b + 1])
                        nc.scalar.activation(
                            out=st.rearrange("p a s -> p (a s)"),
                            in_=st.rearrange("p a s -> p (a s)"),
                            func=AF.Tanh)
                        for i in range(8):
                            b = 8 * bh + i
                            j = b // 4
                            nc.tensor.matmul(
                                ps_sc[32 * j:32 * j + 4, :],
                                va[:, dc, b % 4, :],
                                st[:, i, :],
                                start=(dc == 0 and b % 4 == 0),
                                stop=(dc == DC - 1 and b % 4 == 3),
                                tile_position=(0, 32 * j))

                # softmax per group of 4 rows (partitions 32j..32j+3)
                nm = step_p.tile([128, 1], F32)
                se = step_p.tile([128, 1], F32)
                rs = step_p.tile([128, 1], F32)
                w128 = step_p.tile([128, S], BF16)
                nc.vector.memset(se, 1.0)
                for j in range(4):
                    sl = slice(32 * j, 32 * j + 4)
                    nc.vector.tensor_reduce(
                        out=nm[sl, :], in_=ps_sc[sl, :], axis=mybir.AxisListType.X,
                        op=mybir.AluOpType.max, negate=True)
                    nc.scalar.activation(
                        out=w128[sl, :], in_=ps_sc[sl, :], func=AF.Exp,
                        bias=nm[sl, 0:1], accum_out=se[sl, 0:1])
                nc.vector.reciprocal(rs, se)
                w16 = step_p.tile([BL, S], BF16)
                for j in range(4):
                    sl = slice(32 * j, 32 * j + 4)
                    nc.vector.tensor_scalar_mul(w128[sl, :], w128[sl, :], rs[sl, 0:1])
                    nc.sync.dma_start(out=w16[4 * j:4 * j + 4, :], in_=w128[sl, :])
                nc.sync.dma_start(out=attn_d.ap()[t, :, :], in_=w16)

                if stop_after == "softmax":
                    nc.vector.tensor_copy(hT[nxt], hT[cur])
                    nc.vector.tensor_copy(hTbf[nxt], hTbf[cur])
                    continue
                # wT via plain PE transposes -> zero-interleaved wTz
                for c in range(SC):
                    ps_wT = ps_t_p.tile([128, BL], BF16, name="ps_wT", tag="ps_t")
                    nc.tensor.transpose(
                        ps_wT, w16[:, 128 * c:128 * (c + 1)], id16)
                    nc.vector.tensor_copy(
                        wTz[:, c, 0:BL * BL:BL + 1], ps_wT)

                if stop_after == "wT":
                    nc.vector.tensor_copy(hT[nxt], hT[cur])
                    nc.vector.tensor_copy(hTbf[nxt], hTbf[cur])
                    continue
                # ctx = w^T e, single M=16 accumulation group
                ps_ctx = ps_ctx_p.tile([BL, D], F32)
                for b in range(BL):
                    for sc in range(SC):
                        nc.tensor.matmul(
                            ps_ctx,
                            wTz[:, sc, BL * b:BL * (b + 1)],
                            Ea[:, sc, b, :],
                            start=(b == 0 and sc == 0),
                            stop=(b == BL - 1 and sc == SC - 1))
                ctx_bf = step_p.tile([BL, D], BF16)
                nc.vector.tensor_copy(ctx_bf, ps_ctx)

                # ctxT via PE transpose
                cT = step_p.tile([128, DC, BL], BF16)
                for c in range(DC):
                    ps_cT = ps_t_p.tile([128, BL], BF16, name="ps_cT", tag="ps_t")
                    nc.tensor.transpose(
                        ps_cT, ctx_bf[:, 128 * c:128 * (c + 1)], id16)
                    nc.vector.tensor_copy(cT[:, c, :], ps_cT)

                if stop_after == "ctx":
                    nc.vector.tensor_copy(hT[nxt], hT[cur])
                    nc.vector.tensor_copy(hTbf[nxt], hTbf[cur])
                    continue
                # GRU gate matmuls. Layout ps_g[:, 0:8] = gi+gh for r,z
                # (joint accumulation); [:, 8:12] = gi_n; [:, 12:16] = gh_n.
                ps_g = ps_g_p.tile([128, 16, BL], F32)
                for gc in range(8):
                    g0 = 128 * gc
                    for jc in range(DC):
                        nc.tensor.matmul(
                            ps_g[:, gc, :], WihT[:, jc, g0:g0 + 128], cT[:, jc, :],
                            start=(jc == 0), stop=False)
                    nc.tensor.matmul(
                        ps_g[:, gc, :], WihX[:, g0:g0 + 128],
                        xT[:, BL * t:BL * (t + 1)],
                        start=False, stop=False)
                    for jc in range(DC):
                        nc.tensor.matmul(
                            ps_g[:, gc, :], WhhT[:, jc, g0:g0 + 128],
                            hTbf[cur][:, jc, :],
                            start=False, stop=False)
                    nc.tensor.matmul(
                        ps_g[:, gc, :], bhh[:, g0:g0 + 128], ones1,
                        start=False, stop=True)
                for gc in range(8, 12):
                    g0 = 128 * gc
                    for jc in range(DC):
                        nc.tensor.matmul(
                            ps_g[:, gc, :], WihT[:, jc, g0:g0 + 128], cT[:, jc, :],
                            start=(jc == 0), stop=False)
                    nc.tensor.matmul(
                        ps_g[:, gc, :], WihX[:, g0:g0 + 128],
                        xT[:, BL * t:BL * (t + 1)],
                        start=False, stop=True)
                    for jc in range(DC):
                        nc.tensor.matmul(
                            ps_g[:, 4 + gc, :], WhhT[:, jc, g0:g0 + 128],
                            hTbf[cur][:, jc, :],
                            start=(jc == 0), stop=False)
                    nc.tensor.matmul(
                        ps_g[:, 4 + gc, :], bhh[:, g0:g0 + 128], ones1,
                        start=False, stop=True)

                # gates: r,z = sigmoid(gi+gh); n = tanh(gi_n + r*gh_n)
                rz = step_p.tile([128, 8, BL], F32)
                nc.scalar.activation(
                    out=rz.rearrange("p a b -> p (a b)"),
                    in_=ps_g[:, 0:8, :].rearrange("p a b -> p (a b)"),
                    func=AF.Sigmoid)
                na = step_p.tile([128, DC, BL], F32)
                nc.vector.tensor_tensor(
                    out=na.rearrange("p a b -> p (a b)"),
                    in0=rz[:, 0:4, :].rearrange("p a b -> p (a b)"),
                    in1=ps_g[:, 12:16, :].rearrange("p a b -> p (a b)"),
                    op=mybir.AluOpType.mult)
                nc.vector.tensor_tensor(
                    out=na.rearrange("p a b -> p (a b)"),
                    in0=na.rearrange("p a b -> p (a b)"),
                    in1=ps_g[:, 8:12, :].rearrange("p a b -> p (a b)"),
                    op=mybir.AluOpType.add)
                nc.scalar.activation(
                    out=na.rearrange("p a b -> p (a b)"),
                    in_=na.rearrange("p a b -> p (a b)"), func=AF.Tanh)
                # h_new = n + z*(h - n)
                hmn = step_p.tile([128, DC, BL], F32)
                nc.vector.tensor_tensor(
                    out=hmn.rearrange("p a b -> p (a b)"),
                    in0=hT[cur].rearrange("p a b -> p (a b)"),
                    in1=na.rearrange("p a b -> p (a b)"),
                    op=mybir.AluOpType.subtract)
                nc.vector.tensor_tensor(
                    out=hmn.rearrange("p a b -> p (a b)"),
                    in0=rz[:, 4:8, :].rearrange("p a b -> p (a b)"),
                    in1=hmn.rearrange("p a b -> p (a b)"),
                    op=mybir.AluOpType.mult)
                nc.vector.tensor_tensor(
                    out=hT[nxt].rearrange("p a b -> p (a b)"),
                    in0=na.rearrange("p a b -> p (a b)"),
                    in1=hmn.rearrange("p a b -> p (a b)"),
                    op=mybir.AluOpType.add)
                nc.vector.tensor_copy(hTbf[nxt], hT[nxt])

                # out = W_out h_new + b_out  -> outs staging
                ps_o = ps_t_p.tile([OUT, BL], F32, name="ps_o", tag="ps_t")
                for dc in range(DC):
                    nc.tensor.matmul(
                        ps_o, WoT[:, dc, :], hTbf[nxt][:, dc, :],
                        start=(dc == 0), stop=(dc == DC - 1))
                nc.scalar.add(
                    outs_st[:, BL * t:BL * (t + 1)], ps_o, bo[:, 0:1])

            # final h + outs
            nc.sync.dma_start(out=hT_d.ap(), in_=hT[t_steps % 2])
            nc.sync.dma_start(out=outsT_d.ap(), in_=outs_st)
    return nc


def _prep_inputs(e_all, e_last, target, Wa, ba, Ua, bu, Va_w, Va_b,
                 W_ih, b_ih, W_hh, b_hh, W_out, b_out):
    """Host-side: shard over batch, build transposed bf16 weight layouts."""
    e_all = np.asarray(e_all, np.float32)
    e_last = np.asarray(e_last, np.float32)
    target = np.asarray(target, np.float32)

    # teacher-forcing inputs: step 0 zeros, step t sees target[:, t-1]
    xs = np.concatenate(
        [np.zeros((B, 1, OUT), np.float32), target[:, :T - 1, :]], axis=1)

    # weights (shared by all cores)
    WaT = np.ascontiguousarray(Wa.T).astype(BF)
    UaT = np.ascontiguousarray(Ua.T).astype(BF)
    WihT = np.ascontiguousarray(W_ih[:, :D].T).astype(BF)
    WihX = np.concatenate([W_ih[:, D:].T, b_ih[None, :]], axis=0).astype(BF)
    WhhT = np.ascontiguousarray(W_hh.T).astype(BF)
    bhh = b_hh[None, :].astype(BF)
    va_pat = np.zeros((128, DC, 4, 4), np.float32)
    va_f = np.asarray(Va_w[0], np.float32)
    for dc_ in range(DC):
        for pos in range(4):
            va_pat[:, dc_, pos, pos] = va_f[128 * dc_:128 * (dc_ + 1)]
    va_pat = va_pat.astype(BF)
    idT4 = np.zeros((128, 4), np.float32)
    for j in range(4):
        for i in range(4):
            idT4[32 * j + i, i] = 1.0
    idT4 = idT4.astype(BF)
    id16 = np.eye(16, dtype=np.float32).astype(BF)
    WoT = np.ascontiguousarray(W_out.T).astype(BF)
    bo = np.asarray(b_out, np.float32).reshape(OUT, 1)
    buk = (np.asarray(bu, np.float32) + np.asarray(ba, np.float32)).reshape(
        DC, 128).T.copy()

    shared = dict(WaT=WaT, UaT=UaT, WihT=WihT, WihX=WihX, WhhT=WhhT,
                  bhh=bhh, vap=va_pat, idT4=idT4, id16=id16, WoT=WoT,
                  bo=bo, buk=buk)

    in_maps = []
    for c in range(NC_):
        bsl = slice(BL * c, BL * (c + 1))
        e_sh = e_all[bsl]                              # [BL, S, D]
        eS = np.ascontiguousarray(e_sh.transpose(1, 0, 2)).astype(BF)
        eT = np.ascontiguousarray(e_sh.transpose(2, 0, 1)).astype(BF)
        x_sh = xs[bsl]                                 # [BL, T, 3]
        xT4 = np.concatenate(
            [x_sh.transpose(2, 1, 0).reshape(OUT, T * BL),
             np.ones((1, T * BL), np.float32)], axis=0).astype(BF)
        h0T = np.ascontiguousarray(e_last[0, bsl].T)   # [D, BL] f32
        m = dict(shared)
        m.update(eT=eT, eS=eS, xT=xT4, h0T=h0T)
        in_maps.append(m)
    return in_maps


_NC_CACHE = {}


def kernel(**inputs):
    in_maps = _prep_inputs(**inputs)
    if "nc" not in _NC_CACHE:
        _NC_CACHE["nc"] = build_kernel()
    nc = _NC_CACHE["nc"]
    res = run_bass_kernel_spmd(nc, in_maps, list(range(NC_)))

    d_outputs = np.zeros((B, T, OUT), np.float32)
    hT_full = np.zeros((1, B, D), np.float32)
    cross = np.zeros((B, T, S), np.float32)
    for c in range(NC_):
        r = res.results[c]
        bsl = slice(BL * c, BL * (c + 1))
        # outsT [OUT, T*BL] -> [BL, T, OUT]
        d_outputs[bsl] = r["outsT"].reshape(OUT, T, BL).transpose(2, 1, 0)
        # hTf [128, DC, BL] -> [BL, D]
        hT_full[0, bsl] = r["hTf"].transpose(1, 0, 2).reshape(D, BL).T
        # attn [T, BL, S] bf16 -> [BL, T, S]
        cross[bsl] = r["attn"].astype(np.float32).transpose(1, 0, 2)
    return d_outputs, hT_full, cross
